# revision 1
# baseline (speedup 1.0000x reference)
"""Trainium2 Bass kernel for CpuLstmModel (LSTM over basins).

Reference computation (per timestep t):
    x0    = relu(x_t @ W_in.T + b_in)                    # [G, H]
    gates = x0 @ w_ih.T + b_ih + h @ w_hh.T + b_hh       # [G, 4H]
    i,f,g,o = split(gates, 4)
    c = sigmoid(f)*c + sigmoid(i)*tanh(g)
    h = sigmoid(o)*tanh(c)
    y_t = h @ W_out.T + b_out                            # [G, 1]

Sharding: data-parallel over ngrid (4096 basins) across 8 cores; weights and
h/c state replicated. On-chip layout is feature-major: activations live as
[hid, basins] tiles, basins are the N=512 moving dim.

Perf structure. Measured on this hardware, a sustained PE matmul stream costs
~265 ns/instruction at N=512 moving, nearly independent of dtype — the kernel
is PE *instruction-count* bound (104/step), not FLOP bound. Design:
  * The recurrent matmul h @ w_hh.T runs in fp8 (e4m3) with
    MatmulPerfMode.DoubleRow: one instruction contracts 2 k-slices of 128, so
    the h-side is 32 instructions instead of 64. h and w_hh quantization adds
    ~7e-3 rel err (simulated; 1.01e-2 total on HW) vs the 2e-2 budget; the
    precision-critical input-side matmul x0 @ w_ih.T stays >=bf16. Both gate
    weight matrices are pre-scaled by 32 so the fp8 weights clear the e4m3
    subnormal floor; the gate activations descale by 1/32.
  * Everything else (weights, x0, h, xt) is bf16: same matmul speed as
    float32r here, but half the SBUF/DMA footprint and direct DMA loads.
  * Software pipelining: step t computes linearIn for t+1 and the output
    matmul for t-1, so the tensor engine has ~28 independent matmuls queued
    when it reaches the first h-dependent matmul of a step (covers the
    elementwise tail of the previous step).
  * Gate-group x-side matmuls are hoisted 5 groups ahead of their h-side
    matmuls (PSUM accumulation groups stay open in between; 6 gate banks +
    1 linearIn + 1 output = all 8 PSUM banks).
  * tanh(c_j)/h_j production is deferred one gate-chunk (ew="defer") so the
    tanh, which waits on the DVE c-update, does not bubble the FIFO ACT queue
    ahead of the next chunk's gate activations; the independent i*g product
    runs on the Pool engine.
  * y_t is DMA'd from an SBUF copy of the raw W_out @ h psum row; the scalar
    b_out is added host-side in run().
"""

import numpy as np
import ml_dtypes

import concourse.bass as bass
import concourse.mybir as mybir
import concourse.tile as tile
from concourse import bacc
from concourse.bass import ds, ts
from concourse.bass_utils import run_bass_kernel_spmd

NT = 365
NGRID = 4096
NX = 32
HID = 512
NY = 1
N_CORES = 8
G = NGRID // N_CORES  # basins per core = 512
KC = HID // 128  # hid chunks = 4
NGATE = 4 * HID  # 2048
MC = NGATE // 128  # gate chunks = 16
WS = 32.0  # gate-weight prescale, descaled in the gate activations

F32 = mybir.dt.float32
F32R = mybir.dt.float32r
F8 = mybir.dt.float8e4
BF16 = mybir.dt.bfloat16
AF = mybir.ActivationFunctionType
DR = mybir.MatmulPerfMode.DoubleRow

U = 28  # steps per hardware-loop iteration; EVEN and divides NT-1=364

gate_funcs = [AF.Sigmoid, AF.Sigmoid, AF.Tanh, AF.Sigmoid]  # i, f, g, o


def build_program(
    nt=NT, unroll=U, use_loop=True, hoist=5, order="v2", hmode="f8dr", ew="defer",
    xp2=False, staggered=False,
):
    nc = bacc.Bacc("TRN2", num_devices=N_CORES)

    xt_d = nc.dram_tensor("xtb", [(nt + 2) * NX, G], BF16, kind="ExternalInput").ap()
    wih_d = nc.dram_tensor("wihTb", [HID, NGATE], BF16, kind="ExternalInput").ap()
    if hmode == "f8dr":
        whh8_d = nc.dram_tensor("whh8", [128, KC, NGATE], F8, kind="ExternalInput").ap()
    else:
        whh_d = nc.dram_tensor("whhT", [HID, NGATE], F32, kind="ExternalInput").ap()
    win_d = nc.dram_tensor("winTb", [NX, HID], BF16, kind="ExternalInput").ap()
    wout_d = nc.dram_tensor("woutCb", [128, KC], BF16, kind="ExternalInput").ap()
    bin_d = nc.dram_tensor("binC", [128, KC], F32, kind="ExternalInput").ap()
    bg_d = nc.dram_tensor("bgC", [128, MC], F32, kind="ExternalInput").ap()
    y_d = nc.dram_tensor("y", [nt, G], F32, kind="ExternalOutput").ap()

    with tile.TileContext(nc) as tc:
        with (
            tc.tile_pool(name="const", bufs=1) as cpool,
            tc.tile_pool(name="stag", bufs=2) as stag_pool,
            tc.tile_pool(name="acts", bufs=10) as act_pool,
            tc.tile_pool(name="tmp", bufs=6) as tmp_pool,
            tc.tile_pool(name="state", bufs=1) as state_pool,
            tc.tile_pool(name="ysb", bufs=2) as y_pool,
            tc.tile_pool(name="gpsum", bufs=5 if xp2 else 6, space="PSUM") as gpsum,
            tc.tile_pool(name="xpsum", bufs=2 if xp2 else 1, space="PSUM") as xpsum,
            tc.tile_pool(name="ypsum", bufs=1, space="PSUM") as ypsum,
        ):
            # ---- load weights (bf16 straight from DRAM) ----
            w_ih_r = cpool.tile([128, KC * NGATE], BF16, name="w_ih_r", tag="w_ih_r")
            for k in range(KC):
                nc.gpsimd.dma_start(w_ih_r[:, ts(k, NGATE)], wih_d[ts(k, 128), :])
            if hmode == "f8dr":
                w_hh8 = cpool.tile([128, KC, NGATE], F8, name="w_hh8", tag="w_hh8")
                nc.gpsimd.dma_start(w_hh8[:], whh8_d[:, :, :])
            else:
                w_hh_r = cpool.tile([128, KC * NGATE], F32R, name="w_hh_r", tag="w_hh_r")
                for k in range(KC):
                    st = stag_pool.tile([128, NGATE], F32, name="st", tag="st")
                    nc.gpsimd.dma_start(st[:], whh_d[ts(k, 128), :])
                    nc.vector.tensor_copy(w_hh_r[:, ts(k, NGATE)], st[:])
            w_in_r = cpool.tile([NX, HID], BF16, name="w_in_r", tag="w_in_r")
            nc.gpsimd.dma_start(w_in_r[:], win_d[:, :])
            w_out_r = cpool.tile([128, KC], BF16, name="w_out_r", tag="w_out_r")
            nc.gpsimd.dma_start(w_out_r[:], wout_d[:, :])
            b_in_sb = cpool.tile([128, KC], F32, name="b_in_sb", tag="b_in_sb")
            nc.gpsimd.dma_start(b_in_sb[:], bin_d[:, :])
            b_g_sb = cpool.tile([128, MC], F32, name="b_g_sb", tag="b_g_sb")
            nc.gpsimd.dma_start(b_g_sb[:], bg_d[:, :])

            # ---- persistent state, all ping-pong across step parity ----
            h_f = [
                [state_pool.tile([128, G], BF16, name=f"h{p}{j}", tag=f"h{p}{j}") for j in range(KC)]
                for p in range(2)
            ]
            h8 = [
                state_pool.tile([128, KC, G], F8, name=f"h8{p}", tag=f"h8{p}")
                for p in range(2)
            ]
            c_t = [state_pool.tile([128, G], F32, name=f"c{j}", tag=f"c{j}") for j in range(KC)]
            xt_sb = [
                state_pool.tile([NX, G], BF16, name=f"xts{p}", tag=f"xts{p}") for p in range(2)
            ]
            x0_r = [
                [state_pool.tile([128, G], BF16, name=f"x0{p}{m}", tag=f"x0{p}{m}") for m in range(KC)]
                for p in range(2)
            ]

            def prefetch(t):  # t may be symbolic; parity must be literal
                nc.gpsimd.dma_start(xt_sb[t[1] % 2][:], xt_d[ts(t[0], NX), :])

            def linear_in(t, pe_filler):
                """Emit linearIn for step t; pe_filler: list of thunks emitting
                independent PE work, popped between chunks to cover the
                single-bank xpsum WAR latency."""
                p = t[1] % 2
                for m in range(KC):
                    xps = xpsum.tile([128, G], F32, name="xps", tag="xps")
                    nc.tensor.matmul(
                        xps[:], w_in_r[:, ts(m, 128)], xt_sb[p][:], start=True, stop=True
                    )
                    nc.scalar.activation(
                        x0_r[p][m][:], xps[:], AF.Relu, bias=b_in_sb[:, m : m + 1]
                    )
                    if pe_filler:
                        pe_filler.pop(0)()

            def y_out(t):  # t = (addr, parity) of the step whose h it reads
                p = t[1] % 2
                yps = ypsum.tile([1, G], F32, name="yps", tag="yps")
                for k in range(KC):
                    nc.tensor.matmul(
                        yps[:],
                        w_out_r[:, k : k + 1],
                        h_f[p][k][:],
                        start=(k == 0),
                        stop=(k == KC - 1),
                    )
                # b_out is added host-side; hop through SBUF on the DVE (the
                # ACT queue is the busier one) and DMA out
                y_sb = y_pool.tile([1, G], F32, name="y_sb", tag="y_sb")
                nc.vector.tensor_copy(y_sb[:], yps[:])
                nc.gpsimd.dma_start(y_d[ds(t[0], 1)], y_sb[:])

            def step(t_sym, parity, first, emit_y, emit_prefetch=True):
                """One LSTM step. t_sym: symbolic/int step index, parity: t%2."""
                pp = (parity + 1) % 2  # parity of t-1 / t+1
                hp, hc = h_f[pp], h_f[parity]
                h8p, h8c = h8[pp], h8[parity]

                if emit_prefetch:
                    prefetch((t_sym + 2, parity))

                seq = [(gi * KC + j, j, gi) for j in range(KC) for gi in range(4)]
                groups = {}  # m -> psum tile with x-side accumulated

                def x_phase(idx):
                    m, j, gi = seq[idx]
                    gps = gpsum.tile([128, G], F32, name="gps", tag="gps")
                    for k in range(KC):
                        nc.tensor.matmul(
                            gps[:],
                            w_ih_r[:, ds(k * NGATE + m * 128, 128)],
                            x0_r[parity][k][:],
                            start=(k == 0),
                            stop=(first and k == KC - 1),
                        )
                    groups[idx] = gps

                def h_phase(idx):
                    m, j, gi = seq[idx]
                    gps = groups.pop(idx)
                    if not first:
                        if hmode == "f8dr":
                            for p8 in range(2):
                                nc.tensor.matmul(
                                    gps[:],
                                    w_hh8[:, 2 * p8 : 2 * p8 + 2, ts(m, 128)],
                                    h8p[:, 2 * p8 : 2 * p8 + 2, :],
                                    start=False,
                                    stop=(p8 == 1),
                                    perf_mode=DR,
                                )
                        else:
                            for k in range(KC):
                                nc.tensor.matmul(
                                    gps[:],
                                    w_hh_r[:, ds(k * NGATE + m * 128, 128)],
                                    hp[k][:],
                                    start=False,
                                    stop=(k == KC - 1),
                                )
                    a = act_pool.tile([128, G], F32, name="act", tag="act")
                    nc.scalar.activation(
                        a[:], gps[:], gate_funcs[gi], bias=b_g_sb[:, m : m + 1], scale=1.0 / WS
                    )
                    return a

                # linearIn for t+1 interleaved with early x-phases; y(t-1) goes
                # after them so its last k-chunk (which reads the h written at
                # the very end of step t-1) sits ~28 matmuls into the queue.
                filler = []
                if order == "v1" and emit_y:
                    filler.append(lambda: y_out((t_sym - 1, pp)))
                nh = min(hoist, len(seq))
                for i in range(nh):
                    filler.append(lambda i=i: x_phase(i))
                linear_in((t_sym + 1, pp), filler)
                for f in filler:  # anything not consumed as filler
                    f()
                if order != "v1" and emit_y:
                    y_out((t_sym - 1, pp))

                def ew_front(j, a_i, a_f, a_g):
                    """c update for chunk j (DVE + Pool, no ACT)."""
                    if first:
                        nc.vector.tensor_mul(c_t[j][:], a_i[:], a_g[:])
                    else:
                        t2 = tmp_pool.tile([128, G], F32, name="t2", tag="t2")
                        if ew == "defer":
                            nc.gpsimd.tensor_mul(t2[:], a_i[:], a_g[:])
                        else:
                            nc.vector.tensor_mul(t2[:], a_i[:], a_g[:])
                        t1 = tmp_pool.tile([128, G], F32, name="t1", tag="t1")
                        nc.vector.tensor_mul(t1[:], a_f[:], c_t[j][:])
                        nc.vector.tensor_add(c_t[j][:], t1[:], t2[:])

                def ew_back(j, a_o):
                    """tanh + h for chunk j — deferred so the tanh (which waits
                    on the DVE c-chain) doesn't bubble the FIFO ACT queue ahead
                    of the next chunk's gate activations."""
                    tanc = tmp_pool.tile([128, G], F32, name="tanc", tag="tanc")
                    nc.scalar.activation(tanc[:], c_t[j][:], AF.Tanh)
                    nc.vector.tensor_mul(hc[j][:], a_o[:], tanc[:])
                    if hmode == "f8dr":
                        nc.gpsimd.tensor_copy(h8c[:, j, :], hc[j][:])

                acts = []
                pending = None  # (j, a_o) awaiting deferred tanh/h
                for idx in range(len(seq)):
                    m, j, gi = seq[idx]
                    if order == "v1":
                        if idx + nh < len(seq):
                            x_phase(idx + nh)
                        acts.append(h_phase(idx))
                    else:
                        acts.append(h_phase(idx))
                        if idx + nh < len(seq):
                            x_phase(idx + nh)
                    if gi == 3:  # all four gates of hid-chunk j done
                        a_i, a_f, a_g, a_o = acts
                        acts = []
                        ew_front(j, a_i, a_f, a_g)
                        if ew == "defer":
                            if pending is not None:
                                ew_back(*pending)
                            pending = (j, a_o)
                        else:
                            ew_back(j, a_o)
                if pending is not None:
                    ew_back(*pending)

            # ---- prologue: xt(0), xt(1), linearIn(0) ----
            prefetch((0, 0))
            prefetch((1, 1))
            linear_in((0, 0), [])

            # ---- step 0 (no h recurrence, no y yet) ----
            step(0, 0, first=True, emit_y=False)

            # ---- steps 1..nt-1 ----
            if use_loop:
                assert (nt - 1) % unroll == 0 and unroll % 2 == 0
                with tc.For_i(1, nt, unroll, staggered_reset=staggered) as iv:
                    for u in range(unroll):
                        step(iv + u, (1 + u) % 2, first=False, emit_y=True)
            else:
                for t in range(1, nt):
                    step(t, t % 2, first=False, emit_y=True)

            # ---- epilogue: y(nt-1) ----
            y_out((nt - 1, (nt - 1) % 2))

    nc.compile()
    return nc


def _prep_inputs(nt, inputs, W_in, b_in, w_ih, w_hh, b_ih, b_hh, W_out, b_out):
    f = np.float32
    bf = ml_dtypes.bfloat16
    inputs = np.ascontiguousarray(np.asarray(inputs, f))
    wihT = np.ascontiguousarray((WS * np.asarray(w_ih, f)).T).astype(bf)  # [HID, 4H]
    whh = (WS * np.asarray(w_hh, f)).T  # [HID, 4H]
    whhT = np.ascontiguousarray(whh)
    whh8 = np.ascontiguousarray(
        whh.reshape(KC, 128, NGATE).transpose(1, 0, 2)
    ).astype(ml_dtypes.float8_e4m3)  # [k(128), s(KC), m(4H)]
    winT = np.ascontiguousarray(np.asarray(W_in, f).T).astype(bf)  # [NX, HID]
    woutC = np.ascontiguousarray(np.asarray(W_out, f).reshape(NY, KC, 128)[0].T).astype(bf)
    binC = np.ascontiguousarray(np.asarray(b_in, f).reshape(KC, 128).T)
    bgC = np.ascontiguousarray(
        (np.asarray(b_ih, f) + np.asarray(b_hh, f)).reshape(MC, 128).T
    )
    shared = dict(
        wihTb=wihT, whh8=whh8, whhT=whhT, winTb=winT, woutCb=woutC, binC=binC,
        bgC=bgC,
    )
    in_maps = []
    for c in range(N_CORES):
        xc = inputs[:nt, c * G : (c + 1) * G, :]  # [nt, G, NX]
        xt = np.ascontiguousarray(xc.transpose(0, 2, 1)).reshape(nt * NX, G)
        xt_pad = np.zeros(((nt + 2) * NX, G), bf)
        xt_pad[: nt * NX] = xt.astype(bf)
        in_maps.append({"xtb": xt_pad, **shared})
    return in_maps


def run(inputs_dict, trace=False, nt=NT, unroll=U, use_loop=True, **spmd_kwargs):
    nc = build_program(nt, unroll, use_loop)
    in_maps = _prep_inputs(nt, **inputs_dict)
    res = run_bass_kernel_spmd(
        nc, in_maps, core_ids=list(range(N_CORES)), trace=trace, **spmd_kwargs
    )
    out = np.empty((nt, NGRID, NY), np.float32)
    for c in range(N_CORES):
        out[:, c * G : (c + 1) * G, 0] = res.results[c]["y"]
    out += np.float32(np.asarray(inputs_dict["b_out"]).reshape(-1)[0])
    return out, res


def kernel(**inputs):
    out, _ = run(inputs, trace=False)
    return out



# revision 9
# speedup vs baseline: 1.0835x; 1.0835x over previous
"""Trainium2 Bass kernel for CpuLstmModel (LSTM over basins).

Reference computation (per timestep t):
    x0    = relu(x_t @ W_in.T + b_in)                    # [G, H]
    gates = x0 @ w_ih.T + b_ih + h @ w_hh.T + b_hh       # [G, 4H]
    i,f,g,o = split(gates, 4)
    c = sigmoid(f)*c + sigmoid(i)*tanh(g)
    h = sigmoid(o)*tanh(c)
    y_t = h @ W_out.T + b_out                            # [G, 1]

Sharding: data-parallel over ngrid (4096 basins) across 8 cores; weights and
h/c state replicated. On-chip layout is feature-major: activations live as
[hid, basins] tiles, basins are the N=512 moving dim.

Perf structure. Measured on this hardware, a sustained PE matmul stream costs
~265 ns/instruction at N=512 moving, nearly independent of dtype — the kernel
is PE *instruction-count* bound (104/step), not FLOP bound. Design:
  * The recurrent matmul h @ w_hh.T runs in fp8 (e4m3) with
    MatmulPerfMode.DoubleRow: one instruction contracts 2 k-slices of 128, so
    the h-side is 32 instructions instead of 64. h and w_hh quantization adds
    ~7e-3 rel err (simulated; 1.01e-2 total on HW) vs the 2e-2 budget; the
    precision-critical input-side matmul x0 @ w_ih.T stays >=bf16. Both gate
    weight matrices are pre-scaled by 32 so the fp8 weights clear the e4m3
    subnormal floor; the gate activations descale by 1/32.
  * Everything else (weights, x0, h, xt) is bf16: same matmul speed as
    float32r here, but half the SBUF/DMA footprint and direct DMA loads.
  * Software pipelining: step t computes linearIn for t+1 and the output
    matmul for t-1, so the tensor engine has ~28 independent matmuls queued
    when it reaches the first h-dependent matmul of a step (covers the
    elementwise tail of the previous step).
  * Gate-group x-side matmuls are hoisted 5 groups ahead of their h-side
    matmuls (PSUM accumulation groups stay open in between; 6 gate banks +
    1 linearIn + 1 output = all 8 PSUM banks).
  * tanh(c_j)/h_j production is deferred one gate-chunk (ew="defer") so the
    tanh, which waits on the DVE c-update, does not bubble the FIFO ACT queue
    ahead of the next chunk's gate activations; the independent i*g product
    runs on the Pool engine.
  * y_t is DMA'd from an SBUF copy of the raw W_out @ h psum row; the scalar
    b_out is added host-side in run().
"""

import numpy as np
import ml_dtypes

import concourse.bass as bass
import concourse.mybir as mybir
import concourse.tile as tile
from concourse import bacc
from concourse.bass import ds, ts
from concourse.bass_utils import run_bass_kernel_spmd

NT = 365
NGRID = 4096
NX = 32
HID = 512
NY = 1
N_CORES = 8
G = NGRID // N_CORES  # basins per core = 512
KC = HID // 128  # hid chunks = 4
NGATE = 4 * HID  # 2048
MC = NGATE // 128  # gate chunks = 16
WS = 32.0  # gate-weight prescale, descaled in the gate activations

F32 = mybir.dt.float32
F32R = mybir.dt.float32r
F8 = mybir.dt.float8e4
BF16 = mybir.dt.bfloat16
AF = mybir.ActivationFunctionType
DR = mybir.MatmulPerfMode.DoubleRow

U = 28  # steps per hardware-loop iteration; EVEN and divides NT-1=364

gate_funcs = [AF.Sigmoid, AF.Sigmoid, AF.Tanh, AF.Sigmoid]  # i, f, g, o

# Gates whose x-side matmul stays bf16 (precision-critical). The g gate
# (index 2) feeds c through tanh (slope 1, no sigmoid attenuation) — fp8
# there quadruples the output error (sim: 3.7e-2 vs 1.4e-2 rel).
BFG = (2,)


def build_program(
    nt=NT, unroll=U, use_loop=True, hoist=5, order="v2", hmode="f8dr", ew="defer",
    xp2=False, staggered=False, bfg=BFG,
):
    nc = bacc.Bacc("TRN2", num_devices=N_CORES)
    nbf = len(bfg)

    xt_d = nc.dram_tensor("xtb", [(nt + 2) * NX, G], BF16, kind="ExternalInput").ap()
    wih_d = nc.dram_tensor("wihTb", [HID, nbf * 512], BF16, kind="ExternalInput").ap()
    wih8_d = nc.dram_tensor("wih8", [128, KC, NGATE], F8, kind="ExternalInput").ap()
    if hmode == "f8dr":
        whh8_d = nc.dram_tensor("whh8", [128, KC, NGATE], F8, kind="ExternalInput").ap()
    else:
        whh_d = nc.dram_tensor("whhT", [HID, NGATE], F32, kind="ExternalInput").ap()
    win_d = nc.dram_tensor("winTb", [NX, HID], BF16, kind="ExternalInput").ap()
    wout_d = nc.dram_tensor("woutCb", [128, KC], BF16, kind="ExternalInput").ap()
    bin_d = nc.dram_tensor("binC", [128, KC], F32, kind="ExternalInput").ap()
    bg_d = nc.dram_tensor("bgC", [128, MC], F32, kind="ExternalInput").ap()
    y_d = nc.dram_tensor("y", [nt, G], F32, kind="ExternalOutput").ap()

    with tile.TileContext(nc) as tc:
        with (
            tc.tile_pool(name="const", bufs=1) as cpool,
            tc.tile_pool(name="stag", bufs=2) as stag_pool,
            tc.tile_pool(name="acts", bufs=10) as act_pool,
            tc.tile_pool(name="tmp", bufs=6) as tmp_pool,
            tc.tile_pool(name="state", bufs=1) as state_pool,
            tc.tile_pool(name="ysb", bufs=2) as y_pool,
            tc.tile_pool(name="gpsum", bufs=5 if xp2 else 6, space="PSUM") as gpsum,
            tc.tile_pool(name="xpsum", bufs=2 if xp2 else 1, space="PSUM") as xpsum,
            tc.tile_pool(name="ypsum", bufs=1, space="PSUM") as ypsum,
        ):
            # ---- load weights (bf16 straight from DRAM) ----
            # bf16 copy only for the precision-critical gates in bfg
            w_ih_r = cpool.tile([128, KC * nbf * 512], BF16, name="w_ih_r", tag="w_ih_r")
            for k in range(KC):
                nc.gpsimd.dma_start(
                    w_ih_r[:, ts(k, nbf * 512)], wih_d[ts(k, 128), :]
                )
            w_ih8 = cpool.tile([128, KC, NGATE], F8, name="w_ih8", tag="w_ih8")
            nc.gpsimd.dma_start(w_ih8[:], wih8_d[:, :, :])
            if hmode == "f8dr":
                w_hh8 = cpool.tile([128, KC, NGATE], F8, name="w_hh8", tag="w_hh8")
                nc.gpsimd.dma_start(w_hh8[:], whh8_d[:, :, :])
            else:
                w_hh_r = cpool.tile([128, KC * NGATE], F32R, name="w_hh_r", tag="w_hh_r")
                for k in range(KC):
                    st = stag_pool.tile([128, NGATE], F32, name="st", tag="st")
                    nc.gpsimd.dma_start(st[:], whh_d[ts(k, 128), :])
                    nc.vector.tensor_copy(w_hh_r[:, ts(k, NGATE)], st[:])
            w_in_r = cpool.tile([NX, HID], BF16, name="w_in_r", tag="w_in_r")
            nc.gpsimd.dma_start(w_in_r[:], win_d[:, :])
            w_out_r = cpool.tile([128, KC], BF16, name="w_out_r", tag="w_out_r")
            nc.gpsimd.dma_start(w_out_r[:], wout_d[:, :])
            b_in_sb = cpool.tile([128, KC], F32, name="b_in_sb", tag="b_in_sb")
            nc.gpsimd.dma_start(b_in_sb[:], bin_d[:, :])
            b_g_sb = cpool.tile([128, MC], F32, name="b_g_sb", tag="b_g_sb")
            nc.gpsimd.dma_start(b_g_sb[:], bg_d[:, :])

            # ---- persistent state, all ping-pong across step parity ----
            h_f = [
                [state_pool.tile([128, G], BF16, name=f"h{p}{j}", tag=f"h{p}{j}") for j in range(KC)]
                for p in range(2)
            ]
            h8 = [
                state_pool.tile([128, KC, G], F8, name=f"h8{p}", tag=f"h8{p}")
                for p in range(2)
            ]
            c_t = [state_pool.tile([128, G], F32, name=f"c{j}", tag=f"c{j}") for j in range(KC)]
            xt_sb = [
                state_pool.tile([NX, G], BF16, name=f"xts{p}", tag=f"xts{p}") for p in range(2)
            ]
            x0_r = [
                [state_pool.tile([128, G], BF16, name=f"x0{p}{m}", tag=f"x0{p}{m}") for m in range(KC)]
                for p in range(2)
            ]
            x08 = [
                state_pool.tile([128, KC, G], F8, name=f"x08{p}", tag=f"x08{p}")
                for p in range(2)
            ]

            def prefetch(t):  # t may be symbolic; parity must be literal
                nc.gpsimd.dma_start(xt_sb[t[1] % 2][:], xt_d[ts(t[0], NX), :])

            def linear_in(t, pe_filler):
                """Emit linearIn for step t; pe_filler: list of thunks emitting
                independent PE work, popped between chunks to cover the
                single-bank xpsum WAR latency."""
                p = t[1] % 2
                for m in range(KC):
                    xps = xpsum.tile([128, G], F32, name="xps", tag="xps")
                    nc.tensor.matmul(
                        xps[:], w_in_r[:, ts(m, 128)], xt_sb[p][:], start=True, stop=True
                    )
                    nc.scalar.activation(
                        x0_r[p][m][:], xps[:], AF.Relu, bias=b_in_sb[:, m : m + 1]
                    )
                    nc.gpsimd.tensor_copy(x08[p][:, m, :], x0_r[p][m][:])
                    if pe_filler:
                        pe_filler.pop(0)()

            def y_out(t):  # t = (addr, parity) of the step whose h it reads
                p = t[1] % 2
                yps = ypsum.tile([1, G], F32, name="yps", tag="yps")
                for k in range(KC):
                    nc.tensor.matmul(
                        yps[:],
                        w_out_r[:, k : k + 1],
                        h_f[p][k][:],
                        start=(k == 0),
                        stop=(k == KC - 1),
                    )
                # b_out is added host-side; hop through SBUF on the DVE (the
                # ACT queue is the busier one) and DMA out
                y_sb = y_pool.tile([1, G], F32, name="y_sb", tag="y_sb")
                nc.vector.tensor_copy(y_sb[:], yps[:])
                nc.gpsimd.dma_start(y_d[ds(t[0], 1)], y_sb[:])

            def step(t_sym, parity, first, emit_y, emit_prefetch=True):
                """One LSTM step. t_sym: symbolic/int step index, parity: t%2."""
                pp = (parity + 1) % 2  # parity of t-1 / t+1
                hp, hc = h_f[pp], h_f[parity]
                h8p, h8c = h8[pp], h8[parity]

                if emit_prefetch:
                    prefetch((t_sym + 2, parity))

                seq = [(gi * KC + j, j, gi) for j in range(KC) for gi in range(4)]
                groups = {}  # m -> psum tile with x-side accumulated

                def x_phase(idx):
                    m, j, gi = seq[idx]
                    gps = gpsum.tile([128, G], F32, name="gps", tag="gps")
                    if gi in bfg:
                        base = bfg.index(gi) * 512 + j * 128
                        for k in range(KC):
                            nc.tensor.matmul(
                                gps[:],
                                w_ih_r[:, ds(k * nbf * 512 + base, 128)],
                                x0_r[parity][k][:],
                                start=(k == 0),
                                stop=(first and k == KC - 1),
                            )
                    else:
                        for p8 in range(2):
                            nc.tensor.matmul(
                                gps[:],
                                w_ih8[:, 2 * p8 : 2 * p8 + 2, ts(m, 128)],
                                x08[parity][:, 2 * p8 : 2 * p8 + 2, :],
                                start=(p8 == 0),
                                stop=(first and p8 == 1),
                                perf_mode=DR,
                            )
                    groups[idx] = gps

                def h_phase(idx):
                    m, j, gi = seq[idx]
                    gps = groups.pop(idx)
                    if not first:
                        if hmode == "f8dr":
                            for p8 in range(2):
                                nc.tensor.matmul(
                                    gps[:],
                                    w_hh8[:, 2 * p8 : 2 * p8 + 2, ts(m, 128)],
                                    h8p[:, 2 * p8 : 2 * p8 + 2, :],
                                    start=False,
                                    stop=(p8 == 1),
                                    perf_mode=DR,
                                )
                        else:
                            for k in range(KC):
                                nc.tensor.matmul(
                                    gps[:],
                                    w_hh_r[:, ds(k * NGATE + m * 128, 128)],
                                    hp[k][:],
                                    start=False,
                                    stop=(k == KC - 1),
                                )
                    a = act_pool.tile([128, G], F32, name="act", tag="act")
                    nc.scalar.activation(
                        a[:], gps[:], gate_funcs[gi], bias=b_g_sb[:, m : m + 1], scale=1.0 / WS
                    )
                    return a

                # linearIn for t+1 interleaved with early x-phases; y(t-1) goes
                # after them so its last k-chunk (which reads the h written at
                # the very end of step t-1) sits ~28 matmuls into the queue.
                filler = []
                if order == "v1" and emit_y:
                    filler.append(lambda: y_out((t_sym - 1, pp)))
                nh = min(hoist, len(seq))
                for i in range(nh):
                    filler.append(lambda i=i: x_phase(i))
                linear_in((t_sym + 1, pp), filler)
                for f in filler:  # anything not consumed as filler
                    f()
                if order != "v1" and emit_y:
                    y_out((t_sym - 1, pp))

                def ew_front(j, a_i, a_f, a_g):
                    """c update for chunk j (DVE + Pool, no ACT)."""
                    if first:
                        nc.vector.tensor_mul(c_t[j][:], a_i[:], a_g[:])
                    else:
                        t2 = tmp_pool.tile([128, G], F32, name="t2", tag="t2")
                        if ew == "defer":
                            nc.gpsimd.tensor_mul(t2[:], a_i[:], a_g[:])
                        else:
                            nc.vector.tensor_mul(t2[:], a_i[:], a_g[:])
                        t1 = tmp_pool.tile([128, G], F32, name="t1", tag="t1")
                        nc.vector.tensor_mul(t1[:], a_f[:], c_t[j][:])
                        nc.vector.tensor_add(c_t[j][:], t1[:], t2[:])

                def ew_back(j, a_o):
                    """tanh + h for chunk j — deferred so the tanh (which waits
                    on the DVE c-chain) doesn't bubble the FIFO ACT queue ahead
                    of the next chunk's gate activations."""
                    tanc = tmp_pool.tile([128, G], F32, name="tanc", tag="tanc")
                    nc.scalar.activation(tanc[:], c_t[j][:], AF.Tanh)
                    nc.vector.tensor_mul(hc[j][:], a_o[:], tanc[:])
                    if hmode == "f8dr":
                        nc.gpsimd.tensor_copy(h8c[:, j, :], hc[j][:])

                acts = []
                pending = None  # (j, a_o) awaiting deferred tanh/h
                for idx in range(len(seq)):
                    m, j, gi = seq[idx]
                    if order == "v1":
                        if idx + nh < len(seq):
                            x_phase(idx + nh)
                        acts.append(h_phase(idx))
                    else:
                        acts.append(h_phase(idx))
                        if idx + nh < len(seq):
                            x_phase(idx + nh)
                    if gi == 3:  # all four gates of hid-chunk j done
                        a_i, a_f, a_g, a_o = acts
                        acts = []
                        ew_front(j, a_i, a_f, a_g)
                        if ew == "defer":
                            if pending is not None:
                                ew_back(*pending)
                            pending = (j, a_o)
                        else:
                            ew_back(j, a_o)
                if pending is not None:
                    ew_back(*pending)

            # ---- prologue: xt(0), xt(1), linearIn(0) ----
            prefetch((0, 0))
            prefetch((1, 1))
            linear_in((0, 0), [])

            # ---- step 0 (no h recurrence, no y yet) ----
            step(0, 0, first=True, emit_y=False)

            # ---- steps 1..nt-1 ----
            if use_loop:
                assert (nt - 1) % unroll == 0 and unroll % 2 == 0
                with tc.For_i(1, nt, unroll, staggered_reset=staggered) as iv:
                    for u in range(unroll):
                        step(iv + u, (1 + u) % 2, first=False, emit_y=True)
            else:
                for t in range(1, nt):
                    step(t, t % 2, first=False, emit_y=True)

            # ---- epilogue: y(nt-1) ----
            y_out((nt - 1, (nt - 1) % 2))

    nc.compile()
    return nc


def _prep_inputs(nt, inputs, W_in, b_in, w_ih, w_hh, b_ih, b_hh, W_out, b_out):
    f = np.float32
    bf = ml_dtypes.bfloat16
    inputs = np.ascontiguousarray(np.asarray(inputs, f))
    wihT_f = (WS * np.asarray(w_ih, f)).T  # [HID, 4H]
    # bf16 copy: only the precision-critical gates' columns, per BFG
    wihT = np.ascontiguousarray(
        np.concatenate([wihT_f[:, gi * 512 : (gi + 1) * 512] for gi in BFG], axis=1)
    ).astype(bf)  # [HID, nbf*512]
    wih8 = np.ascontiguousarray(
        wihT_f.reshape(KC, 128, NGATE).transpose(1, 0, 2)
    ).astype(ml_dtypes.float8_e4m3)  # [k(128), s(KC), m(4H)]
    whh = (WS * np.asarray(w_hh, f)).T  # [HID, 4H]
    whhT = np.ascontiguousarray(whh)
    whh8 = np.ascontiguousarray(
        whh.reshape(KC, 128, NGATE).transpose(1, 0, 2)
    ).astype(ml_dtypes.float8_e4m3)  # [k(128), s(KC), m(4H)]
    winT = np.ascontiguousarray(np.asarray(W_in, f).T).astype(bf)  # [NX, HID]
    woutC = np.ascontiguousarray(np.asarray(W_out, f).reshape(NY, KC, 128)[0].T).astype(bf)
    binC = np.ascontiguousarray(np.asarray(b_in, f).reshape(KC, 128).T)
    bgC = np.ascontiguousarray(
        (np.asarray(b_ih, f) + np.asarray(b_hh, f)).reshape(MC, 128).T
    )
    shared = dict(
        wihTb=wihT, wih8=wih8, whh8=whh8, whhT=whhT, winTb=winT, woutCb=woutC,
        binC=binC, bgC=bgC,
    )
    in_maps = []
    for c in range(N_CORES):
        xc = inputs[:nt, c * G : (c + 1) * G, :]  # [nt, G, NX]
        xt = np.ascontiguousarray(xc.transpose(0, 2, 1)).reshape(nt * NX, G)
        xt_pad = np.zeros(((nt + 2) * NX, G), bf)
        xt_pad[: nt * NX] = xt.astype(bf)
        in_maps.append({"xtb": xt_pad, **shared})
    return in_maps


def run(inputs_dict, trace=False, nt=NT, unroll=U, use_loop=True, **spmd_kwargs):
    nc = build_program(nt, unroll, use_loop)
    in_maps = _prep_inputs(nt, **inputs_dict)
    res = run_bass_kernel_spmd(
        nc, in_maps, core_ids=list(range(N_CORES)), trace=trace, **spmd_kwargs
    )
    out = np.empty((nt, NGRID, NY), np.float32)
    for c in range(N_CORES):
        out[:, c * G : (c + 1) * G, 0] = res.results[c]["y"]
    out += np.float32(np.asarray(inputs_dict["b_out"]).reshape(-1)[0])
    return out, res


def kernel(**inputs):
    out, _ = run(inputs, trace=False)
    return out



# revision 41
# speedup vs baseline: 1.5312x; 1.4132x over previous
"""Trainium2 Bass kernel for CpuLstmModel (LSTM over basins).

Reference computation (per timestep t):
    x0    = relu(x_t @ W_in.T + b_in)                    # [G, H]
    gates = x0 @ w_ih.T + b_ih + h @ w_hh.T + b_hh       # [G, 4H]
    i,f,g,o = split(gates, 4)
    c = sigmoid(f)*c + sigmoid(i)*tanh(g)
    h = sigmoid(o)*tanh(c)
    y_t = h @ W_out.T + b_out                            # [G, 1]

Sharding: data-parallel over ngrid (4096 basins) across 8 cores; weights and
h/c state replicated. On-chip layout is feature-major: activations live as
[hid, basins] tiles, basins are the N=512 moving dim.

Perf structure. Measured on this hardware, a sustained N=512 PE matmul stream
costs ~271 ns/instruction in bf16 (~231 inside accumulation chains) and ~233
for an fp8 DoubleRow instruction that contracts 2 k-slices of 128 — the
kernel is PE *instruction-count* bound (80/step ~ 18.8us/step), not FLOP
bound. Design:
  * Both recurrent matmuls run fp8 (e4m3) DoubleRow wherever numerics allow:
    the full h-side h @ w_hh.T (32 instr instead of 64), and the x-side
    x0 @ w_ih.T for the i, f, o gates (24 instead of 48). The g gate's
    x-side stays bf16 (16 instr): g feeds c through tanh (slope 1 — no
    sigmoid attenuation), and quantizing it takes the output error from
    1.3e-2 to 3.7e-2 (budget 2e-2). Gate weights are pre-scaled by 32 to
    clear the e4m3 subnormal floor; gate activations descale by 1/32.
  * linearIn's bias rides in its matmul as a 33rd contraction row of ones
    (b_in is the last row of the staged W_in.T), so the relu runs on the
    otherwise-idle DVE instead of the busy ACT queue.
  * Engine balance per step — PE 18.8us (the roofline), ACT 14.4us (16 gate
    activations + 4 tanh), DVE ~14.6us (relu, fp8 casts of x0 and h, c-state
    add, h mul, y copy), Pool ~7us (i*g and f*c muls). Keeping the fp8 casts
    of x0/h on the DVE rather than Pool is worth ~4.5ms total: the Pool
    queue's latency put them on the critical path of the next step's
    DoubleRow matmuls.
  * Software pipelining: step t computes linearIn for t+1 and the output
    matmul for t-1; x-side gate matmuls are hoisted 5 groups ahead of their
    h-side matmuls (6 gate PSUM banks + 1 linearIn + 1 output = all 8).
  * tanh(c_j)/h_j production is deferred one gate-chunk (ew="defer") so the
    tanh, which waits on the DVE c-update, does not bubble the FIFO ACT queue
    ahead of the next chunk's gate activations.
  * y_t is DMA'd from an SBUF copy of the raw W_out @ h psum row; the scalar
    b_out is added host-side in run().
"""

import numpy as np
import ml_dtypes

import concourse.bass as bass
import concourse.mybir as mybir
import concourse.tile as tile
from concourse import bacc
from concourse.bass import ds, ts
from concourse.bass_utils import run_bass_kernel_spmd

NT = 365
NGRID = 4096
NX = 32
NXP = NX + 1  # xt rows + a ones-row so linearIn's bias rides in the matmul
HID = 512
NY = 1
N_CORES = 8
G = NGRID // N_CORES  # basins per core = 512
KC = HID // 128  # hid chunks = 4
NGATE = 4 * HID  # 2048
MC = NGATE // 128  # gate chunks = 16
WS = 32.0  # gate-weight prescale, descaled in the gate activations

F32 = mybir.dt.float32
F32R = mybir.dt.float32r
F8 = mybir.dt.float8e4
BF16 = mybir.dt.bfloat16
AF = mybir.ActivationFunctionType
DR = mybir.MatmulPerfMode.DoubleRow

U = 28  # steps per hardware-loop iteration; EVEN and divides NT-1=364

gate_funcs = [AF.Sigmoid, AF.Sigmoid, AF.Tanh, AF.Sigmoid]  # i, f, g, o

# Gates whose x-side matmul stays bf16 (precision-critical). The g gate
# (index 2) feeds c through tanh (slope 1, no sigmoid attenuation) — fp8
# there quadruples the output error (sim: 3.7e-2 vs 1.4e-2 rel).
BFG = (2,)


def build_program(
    nt=NT, unroll=U, use_loop=True, hoist=5, order="v2", hmode="f8dr", ew="defer",
    xp2=False, staggered=False, bfg=BFG, cast_eng="vector", relu_eng="dve2",
    noy=False, h8cast_eng="vector", yeng="vector", x8mode="cast",
):
    nc = bacc.Bacc("TRN2", num_devices=N_CORES)
    nbf = len(bfg)

    xt_d = nc.dram_tensor("xtb", [(nt + 2) * NXP, G], BF16, kind="ExternalInput").ap()
    wih_d = nc.dram_tensor("wihTb", [HID, nbf * 512], BF16, kind="ExternalInput").ap()
    wih8_d = nc.dram_tensor("wih8", [128, KC, NGATE], F8, kind="ExternalInput").ap()
    if hmode == "f8dr":
        whh8_d = nc.dram_tensor("whh8", [128, KC, NGATE], F8, kind="ExternalInput").ap()
    else:
        whh_d = nc.dram_tensor("whhT", [HID, NGATE], F32, kind="ExternalInput").ap()
    win_d = nc.dram_tensor("winTb", [NXP, HID], BF16, kind="ExternalInput").ap()
    wout_d = nc.dram_tensor("woutCb", [128, KC], BF16, kind="ExternalInput").ap()
    bin_d = nc.dram_tensor("binC", [128, KC], F32, kind="ExternalInput").ap()
    bg_d = nc.dram_tensor("bgC", [128, MC], F32, kind="ExternalInput").ap()
    # fixed [NT, G] regardless of nt so per-call dispatch cost is constant
    # across nt variants (slope-based per-step decomposition)
    y_d = nc.dram_tensor("y", [NT, G], F32, kind="ExternalOutput").ap()

    with tile.TileContext(nc) as tc:
        with (
            tc.tile_pool(name="const", bufs=1) as cpool,
            tc.tile_pool(name="stag", bufs=2) as stag_pool,
            tc.tile_pool(name="acts", bufs=10) as act_pool,
            tc.tile_pool(name="tmp", bufs=6) as tmp_pool,
            tc.tile_pool(name="state", bufs=1) as state_pool,
            tc.tile_pool(name="ysb", bufs=2) as y_pool,
            tc.tile_pool(name="gpsum", bufs=5 if xp2 else 6, space="PSUM") as gpsum,
            tc.tile_pool(name="xpsum", bufs=2 if xp2 else 1, space="PSUM") as xpsum,
            tc.tile_pool(name="ypsum", bufs=1, space="PSUM") as ypsum,
        ):
            # ---- load weights (bf16 straight from DRAM) ----
            # bf16 copy only for the precision-critical gates in bfg
            if nbf:
                w_ih_r = cpool.tile(
                    [128, KC * nbf * 512], BF16, name="w_ih_r", tag="w_ih_r"
                )
                for k in range(KC):
                    nc.gpsimd.dma_start(
                        w_ih_r[:, ts(k, nbf * 512)], wih_d[ts(k, 128), :]
                    )
            w_ih8 = cpool.tile([128, KC, NGATE], F8, name="w_ih8", tag="w_ih8")
            nc.gpsimd.dma_start(w_ih8[:], wih8_d[:, :, :])
            if hmode == "f8dr":
                w_hh8 = cpool.tile([128, KC, NGATE], F8, name="w_hh8", tag="w_hh8")
                nc.gpsimd.dma_start(w_hh8[:], whh8_d[:, :, :])
            else:
                w_hh_r = cpool.tile([128, KC * NGATE], F32R, name="w_hh_r", tag="w_hh_r")
                for k in range(KC):
                    st = stag_pool.tile([128, NGATE], F32, name="st", tag="st")
                    nc.gpsimd.dma_start(st[:], whh_d[ts(k, 128), :])
                    nc.vector.tensor_copy(w_hh_r[:, ts(k, NGATE)], st[:])
            w_in_r = cpool.tile([NXP, HID], BF16, name="w_in_r", tag="w_in_r")
            nc.gpsimd.dma_start(w_in_r[:], win_d[:, :])
            w_out_r = cpool.tile([128, KC], BF16, name="w_out_r", tag="w_out_r")
            nc.gpsimd.dma_start(w_out_r[:], wout_d[:, :])
            b_in_sb = cpool.tile([128, KC], F32, name="b_in_sb", tag="b_in_sb")
            nc.gpsimd.dma_start(b_in_sb[:], bin_d[:, :])
            b_g_sb = cpool.tile([128, MC], F32, name="b_g_sb", tag="b_g_sb")
            nc.gpsimd.dma_start(b_g_sb[:], bg_d[:, :])
            zero_sb = cpool.tile([128, G], F32, name="zero_sb", tag="zero_sb")
            nc.vector.memset(zero_sb[:], 0.0)

            # ---- persistent state, all ping-pong across step parity ----
            h_f = [
                [state_pool.tile([128, G], BF16, name=f"h{p}{j}", tag=f"h{p}{j}") for j in range(KC)]
                for p in range(2)
            ]
            h8 = [
                state_pool.tile([128, KC, G], F8, name=f"h8{p}", tag=f"h8{p}")
                for p in range(2)
            ]
            c_t = [state_pool.tile([128, G], F32, name=f"c{j}", tag=f"c{j}") for j in range(KC)]
            xt_sb = [
                state_pool.tile([NXP, G], BF16, name=f"xts{p}", tag=f"xts{p}")
                for p in range(2)
            ]
            x0_r = [
                [state_pool.tile([128, G], BF16, name=f"x0{p}{m}", tag=f"x0{p}{m}") for m in range(KC)]
                for p in range(2)
            ]
            x08 = [
                state_pool.tile([128, KC, G], F8, name=f"x08{p}", tag=f"x08{p}")
                for p in range(2)
            ]

            def prefetch(t):  # t may be symbolic; parity must be literal
                nc.gpsimd.dma_start(xt_sb[t[1] % 2][:], xt_d[ts(t[0], NXP), :])

            def relu_emit(p, m, xps):
                if x8mode == "direct":
                    # fp8 copy straight from psum, emitted first so the x-side
                    # DR operand is ready earliest
                    nc.vector.tensor_relu(x08[p][:, m, :], xps[:])
                if relu_eng == "dve":
                    nc.vector.tensor_scalar_max(x0_r[p][m][:], xps[:], 0.0)
                elif relu_eng == "dve2":
                    nc.vector.tensor_relu(x0_r[p][m][:], xps[:])
                elif relu_eng == "dvemax":
                    nc.vector.tensor_max(x0_r[p][m][:], xps[:], zero_sb[:])
                else:
                    # bias already added by the matmul ones-row
                    nc.scalar.activation(x0_r[p][m][:], xps[:], AF.Relu)
                if x8mode != "direct":
                    getattr(nc, cast_eng).tensor_copy(x08[p][:, m, :], x0_r[p][m][:])

            def linear_chunk(t, m):
                """Emit linearIn chunk m for step t."""
                p = t[1] % 2
                xps = xpsum.tile([128, G], F32, name="xps", tag="xps")
                nc.tensor.matmul(
                    xps[:], w_in_r[:, ts(m, 128)], xt_sb[p][:], start=True, stop=True
                )
                relu_emit(p, m, xps)

            def linear_in(t, pe_filler):
                """Emit linearIn for step t; pe_filler: list of thunks emitting
                independent PE work, popped between chunks to cover the
                single-bank xpsum WAR latency."""
                p = t[1] % 2
                for m in range(KC):
                    xps = xpsum.tile([128, G], F32, name="xps", tag="xps")
                    nc.tensor.matmul(
                        xps[:], w_in_r[:, ts(m, 128)], xt_sb[p][:], start=True, stop=True
                    )
                    relu_emit(p, m, xps)
                    if pe_filler:
                        pe_filler.pop(0)()

            def y_out(t):  # t = (addr, parity) of the step whose h it reads
                p = t[1] % 2
                yps = ypsum.tile([1, G], F32, name="yps", tag="yps")
                for k in range(KC):
                    nc.tensor.matmul(
                        yps[:],
                        w_out_r[:, k : k + 1],
                        h_f[p][k][:],
                        start=(k == 0),
                        stop=(k == KC - 1),
                    )
                # b_out is added host-side; hop through SBUF and DMA out
                y_sb = y_pool.tile([1, G], F32, name="y_sb", tag="y_sb")
                if yeng == "scalar":
                    nc.scalar.activation(y_sb[:], yps[:], AF.Copy)
                else:
                    nc.vector.tensor_copy(y_sb[:], yps[:])
                nc.gpsimd.dma_start(y_d[ds(t[0], 1)], y_sb[:])

            def step(t_sym, parity, first, emit_y, emit_prefetch=True):
                """One LSTM step. t_sym: symbolic/int step index, parity: t%2."""
                pp = (parity + 1) % 2  # parity of t-1 / t+1
                hp, hc = h_f[pp], h_f[parity]
                h8p, h8c = h8[pp], h8[parity]

                if emit_prefetch:
                    prefetch((t_sym + 2, parity))

                seq = [(gi * KC + j, j, gi) for j in range(KC) for gi in range(4)]
                groups = {}  # m -> psum tile with x-side accumulated

                def x_phase(idx):
                    m, j, gi = seq[idx]
                    gps = gpsum.tile([128, G], F32, name="gps", tag="gps")
                    if gi in bfg:
                        base = bfg.index(gi) * 512 + j * 128
                        for k in range(KC):
                            nc.tensor.matmul(
                                gps[:],
                                w_ih_r[:, ds(k * nbf * 512 + base, 128)],
                                x0_r[parity][k][:],
                                start=(k == 0),
                                stop=(first and k == KC - 1),
                            )
                    else:
                        for p8 in range(2):
                            nc.tensor.matmul(
                                gps[:],
                                w_ih8[:, 2 * p8 : 2 * p8 + 2, ts(m, 128)],
                                x08[parity][:, 2 * p8 : 2 * p8 + 2, :],
                                start=(p8 == 0),
                                stop=(first and p8 == 1),
                                perf_mode=DR,
                            )
                    groups[idx] = gps

                def h_phase(idx):
                    m, j, gi = seq[idx]
                    gps = groups.pop(idx)
                    if not first:
                        if hmode == "f8dr":
                            for p8 in range(2):
                                nc.tensor.matmul(
                                    gps[:],
                                    w_hh8[:, 2 * p8 : 2 * p8 + 2, ts(m, 128)],
                                    h8p[:, 2 * p8 : 2 * p8 + 2, :],
                                    start=False,
                                    stop=(p8 == 1),
                                    perf_mode=DR,
                                )
                        else:
                            for k in range(KC):
                                nc.tensor.matmul(
                                    gps[:],
                                    w_hh_r[:, ds(k * NGATE + m * 128, 128)],
                                    hp[k][:],
                                    start=False,
                                    stop=(k == KC - 1),
                                )
                    a = act_pool.tile([128, G], F32, name="act", tag="act")
                    nc.scalar.activation(
                        a[:], gps[:], gate_funcs[gi], bias=b_g_sb[:, m : m + 1], scale=1.0 / WS
                    )
                    return a

                # linearIn for t+1 interleaved with early x-phases; y(t-1) goes
                # after them so its last k-chunk (which reads the h written at
                # the very end of step t-1) sits ~28 matmuls into the queue.
                nh = min(hoist, len(seq))
                if order == "v3":
                    # linearIn chunks spread across the gate groups (emitted
                    # inside the idx loop below) so the DVE relu+cast work is
                    # phased with the c-chain instead of clumped at step start
                    for i in range(nh):
                        x_phase(i)
                    if emit_y:
                        y_out((t_sym - 1, pp))
                else:
                    filler = []
                    if order == "v1" and emit_y:
                        filler.append(lambda: y_out((t_sym - 1, pp)))
                    for i in range(nh):
                        filler.append(lambda i=i: x_phase(i))
                    linear_in((t_sym + 1, pp), filler)
                    for f in filler:  # anything not consumed as filler
                        f()
                    if order != "v1" and emit_y:
                        y_out((t_sym - 1, pp))

                def ew_front(j, a_i, a_f, a_g):
                    """c update for chunk j (DVE + Pool, no ACT)."""
                    if first:
                        nc.vector.tensor_mul(c_t[j][:], a_i[:], a_g[:])
                    else:
                        t2 = tmp_pool.tile([128, G], F32, name="t2", tag="t2")
                        if ew == "nodefer":
                            nc.vector.tensor_mul(t2[:], a_i[:], a_g[:])
                        else:
                            nc.gpsimd.tensor_mul(t2[:], a_i[:], a_g[:])
                        t1 = tmp_pool.tile([128, G], F32, name="t1", tag="t1")
                        if ew == "t1pool":
                            nc.gpsimd.tensor_mul(t1[:], a_f[:], c_t[j][:])
                        else:
                            nc.vector.tensor_mul(t1[:], a_f[:], c_t[j][:])
                        nc.vector.tensor_add(c_t[j][:], t1[:], t2[:])

                def ew_back(j, a_o):
                    """tanh + h for chunk j — deferred so the tanh (which waits
                    on the DVE c-chain) doesn't bubble the FIFO ACT queue ahead
                    of the next chunk's gate activations."""
                    tanc = tmp_pool.tile([128, G], F32, name="tanc", tag="tanc")
                    nc.scalar.activation(tanc[:], c_t[j][:], AF.Tanh)
                    if ew == "hmulpool":
                        nc.gpsimd.tensor_mul(hc[j][:], a_o[:], tanc[:])
                    else:
                        nc.vector.tensor_mul(hc[j][:], a_o[:], tanc[:])
                    if hmode == "f8dr":
                        getattr(nc, h8cast_eng).tensor_copy(h8c[:, j, :], hc[j][:])

                lin_pos = {2: 0, 5: 1, 8: 2, 11: 3}  # group idx -> linearIn chunk
                acts = []
                pending = None  # (j, a_o) awaiting deferred tanh/h
                for idx in range(len(seq)):
                    m, j, gi = seq[idx]
                    if order == "v1":
                        if idx + nh < len(seq):
                            x_phase(idx + nh)
                        acts.append(h_phase(idx))
                    else:
                        acts.append(h_phase(idx))
                        if idx + nh < len(seq):
                            x_phase(idx + nh)
                        if order == "v3" and idx in lin_pos:
                            linear_chunk((t_sym + 1, pp), lin_pos[idx])
                    if gi == 3:  # all four gates of hid-chunk j done
                        a_i, a_f, a_g, a_o = acts
                        acts = []
                        ew_front(j, a_i, a_f, a_g)
                        if ew != "nodefer":
                            if pending is not None:
                                ew_back(*pending)
                            pending = (j, a_o)
                        else:
                            ew_back(j, a_o)
                if pending is not None:
                    ew_back(*pending)

            # ---- prologue: xt(0), xt(1), linearIn(0) ----
            prefetch((0, 0))
            prefetch((1, 1))
            linear_in((0, 0), [])

            # ---- step 0 (no h recurrence, no y yet) ----
            step(0, 0, first=True, emit_y=False)

            # ---- steps 1..nt-1 ----
            if use_loop:
                assert (nt - 1) % unroll == 0 and unroll % 2 == 0
                with tc.For_i(1, nt, unroll, staggered_reset=staggered) as iv:
                    for u in range(unroll):
                        step(iv + u, (1 + u) % 2, first=False, emit_y=not noy)
            else:
                for t in range(1, nt):
                    step(t, t % 2, first=False, emit_y=True)

            # ---- epilogue: y(nt-1) ----
            y_out((nt - 1, (nt - 1) % 2))

    nc.compile()
    return nc


def _prep_inputs(nt, inputs, W_in, b_in, w_ih, w_hh, b_ih, b_hh, W_out, b_out):
    f = np.float32
    bf = ml_dtypes.bfloat16
    inputs = np.ascontiguousarray(np.asarray(inputs, f))
    wihT_f = (WS * np.asarray(w_ih, f)).T  # [HID, 4H]
    # bf16 copy: only the precision-critical gates' columns, per BFG
    wihT = np.ascontiguousarray(
        np.concatenate([wihT_f[:, gi * 512 : (gi + 1) * 512] for gi in BFG], axis=1)
    ).astype(bf)  # [HID, nbf*512]
    wih8 = np.ascontiguousarray(
        wihT_f.reshape(KC, 128, NGATE).transpose(1, 0, 2)
    ).astype(ml_dtypes.float8_e4m3)  # [k(128), s(KC), m(4H)]
    whh = (WS * np.asarray(w_hh, f)).T  # [HID, 4H]
    whhT = np.ascontiguousarray(whh)
    whh8 = np.ascontiguousarray(
        whh.reshape(KC, 128, NGATE).transpose(1, 0, 2)
    ).astype(ml_dtypes.float8_e4m3)  # [k(128), s(KC), m(4H)]
    # [NXP, HID]: W_in.T with b_in as the final row (paired with the ones-row
    # appended to xt) so linearIn's bias rides in the matmul
    winT = np.ascontiguousarray(
        np.concatenate(
            [np.asarray(W_in, f).T, np.asarray(b_in, f).reshape(1, HID)], axis=0
        )
    ).astype(bf)
    woutC = np.ascontiguousarray(np.asarray(W_out, f).reshape(NY, KC, 128)[0].T).astype(bf)
    binC = np.ascontiguousarray(np.asarray(b_in, f).reshape(KC, 128).T)
    bgC = np.ascontiguousarray(
        (np.asarray(b_ih, f) + np.asarray(b_hh, f)).reshape(MC, 128).T
    )
    shared = dict(
        wihTb=wihT, wih8=wih8, whh8=whh8, whhT=whhT, winTb=winT, woutCb=woutC,
        binC=binC, bgC=bgC,
    )
    in_maps = []
    ones_row = np.ones((nt, 1, G), f)
    for c in range(N_CORES):
        xc = inputs[:nt, c * G : (c + 1) * G, :]  # [nt, G, NX]
        xt = np.concatenate([xc.transpose(0, 2, 1), ones_row], axis=1)  # [nt,NXP,G]
        xt = np.ascontiguousarray(xt).reshape(nt * NXP, G)
        xt_pad = np.zeros(((nt + 2) * NXP, G), bf)
        xt_pad[: nt * NXP] = xt.astype(bf)
        in_maps.append({"xtb": xt_pad, **shared})
    return in_maps


def run(inputs_dict, trace=False, nt=NT, unroll=U, use_loop=True, **spmd_kwargs):
    nc = build_program(nt, unroll, use_loop)
    in_maps = _prep_inputs(nt, **inputs_dict)
    res = run_bass_kernel_spmd(
        nc, in_maps, core_ids=list(range(N_CORES)), trace=trace, **spmd_kwargs
    )
    out = np.empty((nt, NGRID, NY), np.float32)
    for c in range(N_CORES):
        out[:, c * G : (c + 1) * G, 0] = res.results[c]["y"]
    out += np.float32(np.asarray(inputs_dict["b_out"]).reshape(-1)[0])
    return out, res


def kernel(**inputs):
    out, _ = run(inputs, trace=False)
    return out

